# revision 5
# baseline (speedup 1.0000x reference)
"""Trainium2 Bass kernel for nn_Attention_35107062677619.

Dense transformer attention block (B=2, S=2048, D=4096, 32 Q heads / 8 KV
heads, head_dim 128, RoPE, causal mask) tensor-parallel over 8 NeuronCores.

Sharding: each core owns 4 Q heads + their shared KV head (GQA groups align
with cores), computes projections + RoPE + attention for those heads, then an
on-device AllGather collects the per-core attention outputs and each core
applies its 512-row slice of wo.  The host concatenates the 8 output-feature
slices.

v2 (fused pipeline) layout strategy:
 - single flat loop over the 8 (batch, 512-token) slabs; each iteration emits
   QKV projection + RoPE, attention for the 4 local heads of that q-tile, the
   AllGather for the tile, and the wo-projection of the slab gathered two
   iterations earlier.  One shared 8-bank PSUM budget (2 proj + 2 scores +
   2 AV + 2 wo) keeps all phases in flight so the list scheduler can fill
   every tensor-engine bubble.
 - scores are computed transposed (S_T[k, q]) at 128-column causal
   granularity: cross-diagonal k-tiles are skipped entirely and diagonal
   k-tiles only compute the q >= k columns, with a single shared [128,128]
   triangular exp-mask multiply on the diagonal block.
 - softmax is max-free; the denominator is accumulated OFF the PE by
   DVE/gpsimd tensor_adds over the exp tiles, reduced across partitions with
   gpsimd.partition_all_reduce, inverted with reciprocal_approx_fast.
 - V is transposed to [tok, hd] via DMA-crossbar transposes (no PSUM/PE).
 - x is streamed 3x (one pass per projection-psum pair) to stay inside the
   PSUM budget; weights/x are tile-contiguous bf16 prepared on host.
 - AllGather outputs live in Shared-address-space DRAM (fast HBM-HBM path).
"""

import math
import os

import numpy as np
import ml_dtypes

B = 2
S = 2048
D = 4096
HD = 128
N_HEADS = 32
N_KV = 8
N_CORES = 8
NQH = N_HEADS // N_CORES  # 4 local Q heads
P = 128
SLAB = 512  # token tile (matmul free dim)
KH = D // P  # 32 hidden k-tiles
QKVD = NQH * HD + 2 * HD  # 768 projection output dims
F32 = np.float32
BF16 = ml_dtypes.bfloat16


def _build(nc_cores=N_CORES, s=S):
    """Build the SPMD Bass program (one program, data-parallel over cores)."""
    import concourse.mybir as mybir
    import concourse.tile as tile
    from concourse import bacc, bass_isa

    f32 = mybir.dt.float32
    bf16 = mybir.dt.bfloat16
    EXP = mybir.ActivationFunctionType.Exp

    tok = B * s
    nslab = tok // SLAB  # 8
    sslab = s // SLAB  # 4 slabs per batch
    nkt = s // P  # 16 k-tiles of 128 per batch
    spk = SLAB // P  # 4
    nakt = (nc_cores * NQH * HD) // P  # 32 gathered k-tiles for wo
    C_LAG = 2

    nc = bacc.Bacc("TRN2", target_bir_lowering=False, debug=False,
                   num_devices=nc_cores)

    # x blocks laid out slab-major: [slab, kb, p, t] so per-slab chunks of
    # 8 consecutive kb blocks are contiguous.
    xT = nc.dram_tensor("xT", [nslab * KH * P, SLAB], bf16,
                        kind="ExternalInput")
    wqkvT = nc.dram_tensor("wqkvT", [D, QKVD], bf16, kind="ExternalInput")
    woT = nc.dram_tensor("woT", [nc_cores * NQH * HD, SLAB], bf16,
                         kind="ExternalInput")
    cosq = nc.dram_tensor("cosq", [P, s], bf16, kind="ExternalInput")
    sinq = nc.dram_tensor("sinq", [P, s], bf16, kind="ExternalInput")
    emaskd = nc.dram_tensor("emaskd", [P, P], bf16, kind="ExternalInput")
    outT = nc.dram_tensor("outT", [SLAB, tok], f32, kind="ExternalOutput")

    # [slab][chunk] -> [P, 8, SLAB] view of x
    xT_v = xT.ap().rearrange("(sl c j p) t -> sl c p j t",
                             sl=nslab, c=4, j=8, p=P)
    wqkvT_r = wqkvT.ap().rearrange("(o p) q -> p o q", p=P)
    woT_r = woT.ap().rearrange("(o p) q -> p o q", p=P)

    with tile.TileContext(nc) as tc:
        with (
            tc.tile_pool(name="persist", bufs=1) as persist,
            tc.tile_pool(name="dram", bufs=1, space="DRAM") as dram,
            tc.tile_pool(name="qtp", bufs=2) as qtp,
            tc.tile_pool(name="csp", bufs=2) as csp,
            tc.tile_pool(name="xp", bufs=2) as xp,
            tc.tile_pool(name="rp", bufs=3) as rp,
            tc.tile_pool(name="esp", bufs=8) as esp,
            tc.tile_pool(name="accp", bufs=2) as accp,
            tc.tile_pool(name="op", bufs=3) as op,
            tc.tile_pool(name="gp", bufs=1) as gp,
            tc.tile_pool(name="ocp", bufs=4) as ocp,
            tc.tile_pool(name="psProj", bufs=2, space="PSUM") as psProj,
            tc.tile_pool(name="psS", bufs=2, space="PSUM") as psS,
            tc.tile_pool(name="psAV", bufs=2, space="PSUM") as psAV,
            tc.tile_pool(name="psC", bufs=2, space="PSUM") as psC,
        ):
            cc_in = [dram.tile([NQH * HD, SLAB], bf16, tag=f"cc_in{i}",
                               name=f"cc_in{i}")
                     for i in range(nslab)]
            cc_out = [dram.tile([nc_cores * NQH * HD, SLAB], bf16,
                                tag=f"cc_out{i}", name=f"cc_out{i}",
                                addr_space="Shared")
                      for i in range(nslab)]
            cc_out_r = [t[:].rearrange("(o p) t -> p o t", p=P)
                        for t in cc_out]

            # persistent SBUF
            wqkv_sb = persist.tile([P, KH, QKVD], bf16, tag="wqkv")
            for c in range(8):
                nc.sync.dma_start(wqkv_sb[:, c * 4:(c + 1) * 4, :],
                                  wqkvT_r[:, c * 4:(c + 1) * 4, :])
            emask_sb = persist.tile([P, P], bf16, tag="emaskd")
            nc.sync.dma_start(emask_sb[:], emaskd.ap())
            wo_sb = persist.tile([P, nakt, SLAB], bf16, tag="wo")
            KT = persist.tile([P, tok], bf16, tag="KT")
            V = persist.tile([P, B * nkt, HD], bf16, tag="V")

            PASS_DIMS = [(0, 1), (2, 3), (4, 5)]  # q0..q3, k, v

            def emit_rope(ps, dst, cs_sl, sn_sl, alt, nm):
                h = P // 2
                q_sb = rp.tile([P, SLAB], bf16, tag="qsb", name=f"qsb_{nm}")
                if alt:
                    nc.scalar.copy(q_sb[:], ps[:])
                else:
                    nc.vector.tensor_copy(q_sb[:], ps[:])
                tmp = rp.tile([P, SLAB], bf16, tag="rtmp", name=f"rt_{nm}")
                nc.vector.tensor_copy(tmp[0:h, :], q_sb[h:P, :])
                nc.vector.tensor_copy(tmp[h:P, :], q_sb[0:h, :])
                nc.vector.tensor_mul(tmp[:], tmp[:], sn_sl)
                nc.vector.tensor_mul(dst, q_sb[:], cs_sl)
                nc.vector.tensor_add(dst, dst, tmp[:])

            def emit_C(cs):
                g = gp.tile([P, nakt, SLAB], bf16, tag="g", name=f"g_{cs}")
                for c in range(4):
                    nc.sync.dma_start(g[:, c * 8:(c + 1) * 8, :],
                                      cc_out_r[cs][:, c * 8:(c + 1) * 8, :])
                for od in range(spk):
                    ps = psC.tile([P, SLAB], f32, tag="wops",
                                  name=f"wops_{cs}_{od}")
                    for kb in range(nakt):
                        nc.tensor.matmul(
                            ps[:], wo_sb[:, kb, od * P:(od + 1) * P],
                            g[:, kb, :],
                            start=(kb == 0), stop=(kb == nakt - 1))
                    oc = ocp.tile([P, SLAB], f32, tag="oc",
                                  name=f"oc_{cs}_{od}")
                    if od % 2:
                        nc.scalar.copy(oc[:], ps[:])
                    else:
                        nc.vector.tensor_copy(oc[:], ps[:])
                    nc.sync.dma_start(
                        outT.ap()[od * P:(od + 1) * P,
                                  cs * SLAB:(cs + 1) * SLAB], oc[:])

            for slab in range(nslab):
                b, qt = divmod(slab, sslab)
                t0 = slab * SLAB
                sr = qt * SLAB
                nm = f"{b}_{qt}"

                cs_sl = csp.tile([P, SLAB], bf16, tag="cos", name=f"cs_{nm}")
                sn_sl = csp.tile([P, SLAB], bf16, tag="sin", name=f"sn_{nm}")
                nc.sync.dma_start(cs_sl[:], cosq.ap()[:, sr:sr + SLAB])
                nc.sync.dma_start(sn_sl[:], sinq.ap()[:, sr:sr + SLAB])

                QT = qtp.tile([P, NQH, SLAB], bf16, tag="QT", name=f"QT_{nm}")

                # ---- projection: 3 passes x (4 chunks of 8 kb) ----
                for pi, (d0, d1) in enumerate(PASS_DIMS):
                    ps0 = psProj.tile([P, SLAB], f32, tag="proj",
                                      name=f"pj_{nm}_{pi}_0")
                    ps1 = psProj.tile([P, SLAB], f32, tag="proj",
                                      name=f"pj_{nm}_{pi}_1")
                    for c in range(4):
                        xt = xp.tile([P, 8, SLAB], bf16, tag="x",
                                     name=f"x_{nm}_{pi}_{c}")
                        nc.sync.dma_start(xt[:], xT_v[slab, c])
                        for j in range(8):
                            kb = c * 8 + j
                            nc.tensor.matmul(
                                ps0[:], wqkv_sb[:, kb, d0 * P:(d0 + 1) * P],
                                xt[:, j, :],
                                start=(kb == 0), stop=(kb == KH - 1))
                            nc.tensor.matmul(
                                ps1[:], wqkv_sb[:, kb, d1 * P:(d1 + 1) * P],
                                xt[:, j, :],
                                start=(kb == 0), stop=(kb == KH - 1))
                    for di, (ps, d) in enumerate(((ps0, d0), (ps1, d1))):
                        if d < NQH:
                            emit_rope(ps, QT[:, d, :], cs_sl[:], sn_sl[:],
                                      di == 0, f"{nm}_q{d}")
                        elif d == NQH:  # K
                            emit_rope(ps, KT[:, t0:t0 + SLAB], cs_sl[:],
                                      sn_sl[:], True, f"{nm}_k")
                        else:  # V: copy out + DMA-crossbar transpose
                            vtmp = rp.tile([P, SLAB], bf16, tag="vtmp",
                                           name=f"vt_{nm}")
                            nc.vector.tensor_copy(vtmp[:], ps[:])
                            for jj in range(spk):
                                nc.sync.dma_start(
                                    V[:, b * nkt + qt * spk + jj, :],
                                    vtmp[:, jj * P:(jj + 1) * P],
                                    transpose=True)

                # ---- attention for the 4 local heads of this q-tile ----
                nkb = spk * (qt + 1)
                for l in range(NQH):
                    pfx = f"{nm}_{l}"
                    acc_d = accp.tile([P, SLAB], f32, tag="accd",
                                      name=f"accd_{pfx}")
                    acc_p = accp.tile([P, SLAB], f32, tag="accp",
                                      name=f"accp_{pfx}")
                    nc.vector.memset(acc_d[:], 0.0)
                    nc.gpsimd.memset(acc_p[:], 0.0)
                    av = psAV.tile([P, SLAB], f32, tag="av", name=f"av_{pfx}")
                    for kb in range(nkb):
                        j = kb - (nkb - spk)
                        qoff = j * P if j > 0 else 0
                        w = SLAB - qoff
                        stg = psS.tile([P, SLAB], f32, tag="st",
                                       name=f"st_{pfx}_{kb}")
                        nc.tensor.matmul(
                            stg[:, 0:w],
                            KT[:, b * s + kb * P:b * s + (kb + 1) * P],
                            QT[:, l, qoff:SLAB],
                            start=True, stop=True)
                        es = esp.tile([P, SLAB], bf16, tag="es",
                                      name=f"es_{pfx}_{kb}")
                        nc.scalar.activation(es[:, 0:w], stg[:, 0:w], EXP)
                        if j >= 0:
                            nc.vector.tensor_mul(es[:, 0:P], es[:, 0:P],
                                                 emask_sb[:])
                        if kb % 3 != 2:
                            nc.vector.tensor_add(acc_d[:, qoff:SLAB],
                                                 acc_d[:, qoff:SLAB],
                                                 es[:, 0:w])
                        else:
                            nc.gpsimd.tensor_add(acc_p[:, qoff:SLAB],
                                                 acc_p[:, qoff:SLAB],
                                                 es[:, 0:w])
                        nc.tensor.matmul(
                            av[:, qoff:SLAB], V[:, b * nkt + kb, :],
                            es[:, 0:w],
                            start=(kb == 0), stop=(kb == nkb - 1),
                            skip_group_check=True)
                    o_u = op.tile([P, SLAB], bf16, tag="ou", name=f"ou_{pfx}")
                    if l % 2:
                        nc.scalar.copy(o_u[:], av[:])
                    else:
                        nc.vector.tensor_copy(o_u[:], av[:])
                    nc.vector.tensor_add(acc_d[:], acc_d[:], acc_p[:])
                    accr = accp.tile([P, SLAB], f32, tag="accr",
                                     name=f"accr_{pfx}")
                    nc.gpsimd.partition_all_reduce(
                        accr[:], acc_d[:], channels=P,
                        reduce_op=bass_isa.ReduceOp.add)
                    nc.vector.reciprocal_approx_fast(acc_p[:], accr[:])
                    o = op.tile([P, SLAB], bf16, tag="o", name=f"o_{pfx}")
                    nc.vector.tensor_mul(o[:], o_u[:], acc_p[:])
                    nc.sync.dma_start(cc_in[slab][l * HD:(l + 1) * HD, :],
                                      o[:])

                # AllGather this (batch, q-tile) across cores
                nc.gpsimd.collective_compute(
                    "AllGather",
                    mybir.AluOpType.bypass,
                    ins=[cc_in[slab].opt()],
                    outs=[cc_out[slab].opt()],
                    replica_groups=[list(range(nc_cores))],
                )

                # wo weights load, spread over the first two iterations
                if slab < 2:
                    nc.sync.dma_start(
                        wo_sb[:, slab * 16:(slab + 1) * 16, :],
                        woT_r[:, slab * 16:(slab + 1) * 16, :])

                # ---- wo projection of the slab gathered C_LAG iters ago ----
                if slab >= C_LAG:
                    emit_C(slab - C_LAG)
            for cs in range(nslab - C_LAG, nslab):
                emit_C(cs)

    nc.compile()
    return nc


def _prep_inputs(x, wq, wk, wv, wo, freqs_cos, freqs_sin, mask,
                 nc_cores=N_CORES, s=S):
    """Host-side sharding + layout prep. Returns per-core input maps."""
    tok = B * s
    x = np.asarray(x, F32)
    nslab = tok // SLAB
    # slab-major tiled layout: [slab, kb, p, t]
    xT = np.ascontiguousarray(
        x.reshape(nslab, SLAB, D // P, P).transpose(0, 2, 3, 1)
    ).astype(BF16).reshape(nslab * D // P * P, SLAB)

    # de-interleave permutation within a head: [x0_0..x0_63, x1_0..x1_63]
    perm = np.concatenate([np.arange(0, HD, 2), np.arange(1, HD, 2)])

    cos = np.asarray(freqs_cos, F32)  # [s, 64]
    sin = np.asarray(freqs_sin, F32)
    cosq = np.ascontiguousarray(
        np.concatenate([cos.T, cos.T], axis=0)).astype(BF16)
    # the shifted partner is multiplied by the DESTINATION row's sin entry:
    # o_top = x0*c - x1*s  -> top rows carry -sin
    # o_bot = x1*c + x0*s  -> bottom rows carry +sin
    sinq = np.ascontiguousarray(
        np.concatenate([-sin.T, sin.T], axis=0)).astype(BF16)

    # one shared [k, q] lower-triangular (incl diag) 0/1 mask for the
    # 128x128 diagonal blocks
    emaskd = np.ascontiguousarray(
        np.tril(np.ones((P, P), dtype=F32)).T).astype(BF16)

    scale = 1.0 / math.sqrt(HD)
    in_maps = []
    for c in range(nc_cores):
        wq_c = np.asarray(wq, F32)[c * NQH * HD:(c + 1) * NQH * HD]  # [512, D]
        wq_c = (wq_c.reshape(NQH, HD, D)[:, perm, :] * scale).reshape(
            NQH * HD, D)
        wk_c = np.asarray(wk, F32)[c * HD:(c + 1) * HD][perm, :]  # [128, D]
        wv_c = np.asarray(wv, F32)[c * HD:(c + 1) * HD]  # [128, D]
        wqkvT = np.ascontiguousarray(
            np.concatenate([wq_c, wk_c, wv_c], axis=0).T).astype(BF16)
        woT = np.ascontiguousarray(
            np.asarray(wo, F32)[c * SLAB:(c + 1) * SLAB].T).astype(BF16)
        in_maps.append({
            "xT": xT,
            "wqkvT": wqkvT,
            "woT": woT,
            "cosq": cosq,
            "sinq": sinq,
            "emaskd": emaskd,
        })
    return in_maps


_NC_CACHE = {}


def _get_nc(nc_cores=N_CORES, s=S):
    key = (nc_cores, s)
    if key not in _NC_CACHE:
        _NC_CACHE[key] = _build(nc_cores, s)
    return _NC_CACHE[key]


def _assemble(results, nc_cores=N_CORES, s=S):
    out = np.empty((B, s, nc_cores * SLAB), dtype=F32)
    for c in range(nc_cores):
        oT = results[c]["outT"]  # [512, tok]
        out[:, :, c * SLAB:(c + 1) * SLAB] = oT.T.reshape(B, s, SLAB)
    return out


def _run(inputs, trace=False, nc_cores=N_CORES, s=S):
    from concourse.bass_utils import run_bass_kernel_spmd

    nc = _get_nc(nc_cores, s)
    in_maps = _prep_inputs(**inputs, nc_cores=nc_cores, s=s)
    res = run_bass_kernel_spmd(nc, in_maps, core_ids=list(range(nc_cores)),
                               trace=trace)
    return _assemble(res.results, nc_cores, s), res


def kernel(x, wq, wk, wv, wo, freqs_cos, freqs_sin, mask):
    out, _ = _run(dict(x=x, wq=wq, wk=wk, wv=wv, wo=wo,
                       freqs_cos=freqs_cos, freqs_sin=freqs_sin, mask=mask),
                  trace=bool(int(os.environ.get("KERNEL_TRACE", "0"))))
    return out


# revision 12
# speedup vs baseline: 1.0909x; 1.0909x over previous
"""Trainium2 Bass kernel for nn_Attention_35107062677619.

Dense transformer attention block (B=2, S=2048, D=4096, 32 Q heads / 8 KV
heads, head_dim 128, RoPE, causal mask) tensor-parallel over 8 NeuronCores.

Sharding: each core owns 4 Q heads + their shared KV head (GQA groups align
with cores), computes projections + RoPE + attention for those heads, then an
on-device AllGather collects the per-core attention outputs and each core
applies its 512-row slice of wo.  The host concatenates the 8 output-feature
slices.

v2 (fused pipeline) layout strategy:
 - single flat loop over the 8 (batch, 512-token) slabs; each iteration emits
   QKV projection + RoPE, attention for the 4 local heads of that q-tile, the
   AllGather for the tile, and the wo-projection of the slab gathered two
   iterations earlier.  One shared 8-bank PSUM budget (2 proj + 2 scores +
   2 AV + 2 wo) keeps all phases in flight so the list scheduler can fill
   every tensor-engine bubble.
 - scores are computed transposed (S_T[k, q]) at 128-column causal
   granularity: cross-diagonal k-tiles are skipped entirely and diagonal
   k-tiles only compute the q >= k columns, with a single shared [128,128]
   triangular exp-mask multiply on the diagonal block.
 - softmax is max-free; the denominator is accumulated OFF the PE by
   DVE/gpsimd tensor_adds over the exp tiles, reduced across partitions with
   gpsimd.partition_all_reduce, inverted with reciprocal_approx_fast.
 - V is transposed to [tok, hd] via DMA-crossbar transposes (no PSUM/PE).
 - x is streamed 3x (one pass per projection-psum pair) to stay inside the
   PSUM budget; weights/x are tile-contiguous bf16 prepared on host.
 - AllGather outputs live in Shared-address-space DRAM (fast HBM-HBM path).
"""

import math
import os

import numpy as np
import ml_dtypes

B = 2
S = 2048
D = 4096
HD = 128
N_HEADS = 32
N_KV = 8
N_CORES = 8
NQH = N_HEADS // N_CORES  # 4 local Q heads
P = 128
SLAB = 512  # token tile (matmul free dim)
KH = D // P  # 32 hidden k-tiles
QKVD = NQH * HD + 2 * HD  # 768 projection output dims
F32 = np.float32
BF16 = ml_dtypes.bfloat16


def _build(nc_cores=N_CORES, s=S):
    """Build the SPMD Bass program (one program, data-parallel over cores)."""
    import concourse.mybir as mybir
    import concourse.tile as tile
    from concourse import bacc, bass_isa

    f32 = mybir.dt.float32
    bf16 = mybir.dt.bfloat16
    EXP = mybir.ActivationFunctionType.Exp

    tok = B * s
    nslab = tok // SLAB  # 8
    sslab = s // SLAB  # 4 slabs per batch
    nkt = s // P  # 16 k-tiles of 128 per batch
    spk = SLAB // P  # 4
    nakt = (nc_cores * NQH * HD) // P  # 32 gathered k-tiles for wo
    C_LAG = 2

    nc = bacc.Bacc("TRN2", target_bir_lowering=False, debug=False,
                   num_devices=nc_cores)

    # x blocks laid out slab-major: [slab, kb, p, t] so per-slab chunks of
    # 8 consecutive kb blocks are contiguous.
    xT = nc.dram_tensor("xT", [nslab * KH * P, SLAB], bf16,
                        kind="ExternalInput")
    wqkvT = nc.dram_tensor("wqkvT", [D, QKVD], bf16, kind="ExternalInput")
    woT = nc.dram_tensor("woT", [nc_cores * NQH * HD, SLAB], bf16,
                         kind="ExternalInput")
    cosq = nc.dram_tensor("cosq", [P, s], bf16, kind="ExternalInput")
    sinq = nc.dram_tensor("sinq", [P, s], bf16, kind="ExternalInput")
    emaskd = nc.dram_tensor("emaskd", [P, P], bf16, kind="ExternalInput")
    outT = nc.dram_tensor("outT", [SLAB, tok], f32, kind="ExternalOutput")

    # [slab][chunk] -> [P, 4, SLAB] view of x (8 chunks per slab)
    xT_v = xT.ap().rearrange("(sl c j p) t -> sl c p j t",
                             sl=nslab, c=8, j=4, p=P)
    wqkvT_r = wqkvT.ap().rearrange("(o p) q -> p o q", p=P)
    woT_r = woT.ap().rearrange("(o p) q -> p o q", p=P)

    with tile.TileContext(nc) as tc:
        with (
            tc.tile_pool(name="persist", bufs=1) as persist,
            tc.tile_pool(name="dram", bufs=1, space="DRAM") as dram,
            tc.tile_pool(name="qtp", bufs=2) as qtp,
            tc.tile_pool(name="xp", bufs=1) as xp,
            tc.tile_pool(name="rp", bufs=2) as rp,
            tc.tile_pool(name="esp", bufs=6) as esp,
            tc.tile_pool(name="accp", bufs=2) as accp,
            tc.tile_pool(name="op", bufs=2) as op,
            tc.tile_pool(name="gp", bufs=1) as gp,
            tc.tile_pool(name="ocp", bufs=2) as ocp,
            tc.tile_pool(name="psProj", bufs=2, space="PSUM") as psProj,
            tc.tile_pool(name="psS", bufs=2, space="PSUM") as psS,
            tc.tile_pool(name="psAV", bufs=2, space="PSUM") as psAV,
            tc.tile_pool(name="psC", bufs=2, space="PSUM") as psC,
        ):
            cc_in = [dram.tile([NQH * HD, SLAB], bf16, tag=f"cc_in{i}",
                               name=f"cc_in{i}")
                     for i in range(nslab)]
            cc_out = [dram.tile([nc_cores * NQH * HD, SLAB], bf16,
                                tag=f"cc_out{i}", name=f"cc_out{i}",
                                addr_space="Shared")
                      for i in range(nslab)]
            cc_out_r = [t[:].rearrange("(o p) t -> p o t", p=P)
                        for t in cc_out]

            # persistent SBUF
            wqkv_sb = persist.tile([P, KH, QKVD], bf16, tag="wqkv")
            for c in range(8):
                nc.sync.dma_start(wqkv_sb[:, c * 4:(c + 1) * 4, :],
                                  wqkvT_r[:, c * 4:(c + 1) * 4, :])
            emask_sb = persist.tile([P, P], bf16, tag="emaskd")
            nc.sync.dma_start(emask_sb[:], emaskd.ap())
            wo_sb = persist.tile([P, nakt, SLAB], bf16, tag="wo")
            KT = persist.tile([P, tok], bf16, tag="KT")
            V = persist.tile([P, B * nkt, HD], bf16, tag="V")
            cos_sb = persist.tile([P, s], bf16, tag="cos")
            sin_sb = persist.tile([P, s], bf16, tag="sin")
            nc.sync.dma_start(cos_sb[:], cosq.ap())
            nc.sync.dma_start(sin_sb[:], sinq.ap())

            PASS_DIMS = [(0, 1), (2, 3), (4, 5)]  # q0..q3, k, v

            def emit_rope(ps, dst, cs_sl, sn_sl, alt, nm):
                h = P // 2
                q_sb = rp.tile([P, SLAB], bf16, tag="qsb", name=f"qsb_{nm}")
                if alt:
                    nc.scalar.copy(q_sb[:], ps[:])
                else:
                    nc.vector.tensor_copy(q_sb[:], ps[:])
                tmp = rp.tile([P, SLAB], bf16, tag="rtmp", name=f"rt_{nm}")
                nc.vector.tensor_copy(tmp[0:h, :], q_sb[h:P, :])
                nc.vector.tensor_copy(tmp[h:P, :], q_sb[0:h, :])
                nc.vector.tensor_mul(tmp[:], tmp[:], sn_sl)
                nc.vector.tensor_mul(dst, q_sb[:], cs_sl)
                nc.vector.tensor_add(dst, dst, tmp[:])

            def emit_C(cs):
                g = gp.tile([P, nakt, SLAB], bf16, tag="g", name=f"g_{cs}")
                for c in range(8):
                    nc.sync.dma_start(g[:, c * 4:(c + 1) * 4, :],
                                      cc_out_r[cs][:, c * 4:(c + 1) * 4, :])
                for od in range(spk):
                    ps = psC.tile([P, SLAB], f32, tag="wops",
                                  name=f"wops_{cs}_{od}")
                    for kb in range(nakt):
                        nc.tensor.matmul(
                            ps[:], wo_sb[:, kb, od * P:(od + 1) * P],
                            g[:, kb, :],
                            start=(kb == 0), stop=(kb == nakt - 1))
                    oc = ocp.tile([P, SLAB], f32, tag="oc",
                                  name=f"oc_{cs}_{od}")
                    if od % 2:
                        nc.scalar.copy(oc[:], ps[:])
                    else:
                        nc.vector.tensor_copy(oc[:], ps[:])
                    nc.sync.dma_start(
                        outT.ap()[od * P:(od + 1) * P,
                                  cs * SLAB:(cs + 1) * SLAB], oc[:])

            for slab in range(nslab):
                b, qt = divmod(slab, sslab)
                t0 = slab * SLAB
                sr = qt * SLAB
                nm = f"{b}_{qt}"

                cs_sl = cos_sb[:, sr:sr + SLAB]
                sn_sl = sin_sb[:, sr:sr + SLAB]

                QT = qtp.tile([P, NQH, SLAB], bf16, tag="QT", name=f"QT_{nm}")

                # ---- projection: x resident per slab, 3 psum-pair passes ----
                xsb = xp.tile([P, KH, SLAB], bf16, tag="x", name=f"x_{nm}")
                for c in range(8):
                    nc.sync.dma_start(xsb[:, c * 4:(c + 1) * 4, :],
                                      xT_v[slab, c])
                for pi, (d0, d1) in enumerate(PASS_DIMS):
                    ps0 = psProj.tile([P, SLAB], f32, tag="proj",
                                      name=f"pj_{nm}_{pi}_0")
                    ps1 = psProj.tile([P, SLAB], f32, tag="proj",
                                      name=f"pj_{nm}_{pi}_1")
                    for kb in range(KH):
                        nc.tensor.matmul(
                            ps0[:], wqkv_sb[:, kb, d0 * P:(d0 + 1) * P],
                            xsb[:, kb, :],
                            start=(kb == 0), stop=(kb == KH - 1))
                        nc.tensor.matmul(
                            ps1[:], wqkv_sb[:, kb, d1 * P:(d1 + 1) * P],
                            xsb[:, kb, :],
                            start=(kb == 0), stop=(kb == KH - 1))
                    for di, (ps, d) in enumerate(((ps0, d0), (ps1, d1))):
                        if d < NQH:
                            emit_rope(ps, QT[:, d, :], cs_sl, sn_sl,
                                      di == 0, f"{nm}_q{d}")
                        elif d == NQH:  # K
                            emit_rope(ps, KT[:, t0:t0 + SLAB], cs_sl,
                                      sn_sl, True, f"{nm}_k")
                        else:  # V: copy out + DMA-crossbar transpose
                            vtmp = rp.tile([P, SLAB], bf16, tag="vtmp",
                                           name=f"vt_{nm}")
                            nc.vector.tensor_copy(vtmp[:], ps[:])
                            for jj in range(spk):
                                nc.sync.dma_start(
                                    V[:, b * nkt + qt * spk + jj, :],
                                    vtmp[:, jj * P:(jj + 1) * P],
                                    transpose=True)

                # ---- attention for the 4 local heads of this q-tile ----
                nkb = spk * (qt + 1)
                for l in range(NQH):
                    pfx = f"{nm}_{l}"
                    acc_d = accp.tile([P, SLAB], f32, tag="accd",
                                      name=f"accd_{pfx}")
                    acc_p = accp.tile([P, SLAB], f32, tag="accp",
                                      name=f"accp_{pfx}")
                    nc.vector.memset(acc_d[:], 0.0)
                    nc.gpsimd.memset(acc_p[:], 0.0)
                    av = psAV.tile([P, SLAB], f32, tag="av", name=f"av_{pfx}")
                    for kb in range(nkb):
                        j = kb - (nkb - spk)
                        qoff = j * P if j > 0 else 0
                        w = SLAB - qoff
                        stg = psS.tile([P, SLAB], f32, tag="st",
                                       name=f"st_{pfx}_{kb}")
                        nc.tensor.matmul(
                            stg[:, 0:w],
                            KT[:, b * s + kb * P:b * s + (kb + 1) * P],
                            QT[:, l, qoff:SLAB],
                            start=True, stop=True)
                        es = esp.tile([P, SLAB], bf16, tag="es",
                                      name=f"es_{pfx}_{kb}")
                        nc.scalar.activation(es[:, 0:w], stg[:, 0:w], EXP)
                        if j >= 0:
                            nc.vector.tensor_mul(es[:, 0:P], es[:, 0:P],
                                                 emask_sb[:])
                        if kb % 3 != 2:
                            nc.vector.tensor_add(acc_d[:, qoff:SLAB],
                                                 acc_d[:, qoff:SLAB],
                                                 es[:, 0:w])
                        else:
                            nc.gpsimd.tensor_add(acc_p[:, qoff:SLAB],
                                                 acc_p[:, qoff:SLAB],
                                                 es[:, 0:w])
                        nc.tensor.matmul(
                            av[:, qoff:SLAB], V[:, b * nkt + kb, :],
                            es[:, 0:w],
                            start=(kb == 0), stop=(kb == nkb - 1),
                            skip_group_check=True)
                    o_u = op.tile([P, SLAB], bf16, tag="ou", name=f"ou_{pfx}")
                    if l % 2:
                        nc.scalar.copy(o_u[:], av[:])
                    else:
                        nc.vector.tensor_copy(o_u[:], av[:])
                    nc.vector.tensor_add(acc_d[:], acc_d[:], acc_p[:])
                    accr = accp.tile([P, SLAB], f32, tag="accr",
                                     name=f"accr_{pfx}", bufs=1)
                    nc.gpsimd.partition_all_reduce(
                        accr[:], acc_d[:], channels=P,
                        reduce_op=bass_isa.ReduceOp.add)
                    nc.vector.reciprocal_approx_fast(acc_p[:], accr[:])
                    o = op.tile([P, SLAB], bf16, tag="o", name=f"o_{pfx}")
                    nc.vector.tensor_mul(o[:], o_u[:], acc_p[:])
                    nc.sync.dma_start(cc_in[slab][l * HD:(l + 1) * HD, :],
                                      o[:])

                # AllGather this (batch, q-tile) across cores
                nc.gpsimd.collective_compute(
                    "AllGather",
                    mybir.AluOpType.bypass,
                    ins=[cc_in[slab].opt()],
                    outs=[cc_out[slab].opt()],
                    replica_groups=[list(range(nc_cores))],
                )

                # wo weights load, spread over the first two iterations
                if slab < 2:
                    nc.sync.dma_start(
                        wo_sb[:, slab * 16:(slab + 1) * 16, :],
                        woT_r[:, slab * 16:(slab + 1) * 16, :])

                # ---- wo projection of the slab gathered C_LAG iters ago ----
                if slab >= C_LAG:
                    emit_C(slab - C_LAG)
            for cs in range(nslab - C_LAG, nslab):
                emit_C(cs)

    nc.compile()
    return nc


def _prep_inputs(x, wq, wk, wv, wo, freqs_cos, freqs_sin, mask,
                 nc_cores=N_CORES, s=S):
    """Host-side sharding + layout prep. Returns per-core input maps."""
    tok = B * s
    x = np.asarray(x, F32)
    nslab = tok // SLAB
    # slab-major tiled layout: [slab, kb, p, t]
    xT = np.ascontiguousarray(
        x.reshape(nslab, SLAB, D // P, P).transpose(0, 2, 3, 1)
    ).astype(BF16).reshape(nslab * D // P * P, SLAB)

    # de-interleave permutation within a head: [x0_0..x0_63, x1_0..x1_63]
    perm = np.concatenate([np.arange(0, HD, 2), np.arange(1, HD, 2)])

    cos = np.asarray(freqs_cos, F32)  # [s, 64]
    sin = np.asarray(freqs_sin, F32)
    cosq = np.ascontiguousarray(
        np.concatenate([cos.T, cos.T], axis=0)).astype(BF16)
    # the shifted partner is multiplied by the DESTINATION row's sin entry:
    # o_top = x0*c - x1*s  -> top rows carry -sin
    # o_bot = x1*c + x0*s  -> bottom rows carry +sin
    sinq = np.ascontiguousarray(
        np.concatenate([-sin.T, sin.T], axis=0)).astype(BF16)

    # one shared [k, q] lower-triangular (incl diag) 0/1 mask for the
    # 128x128 diagonal blocks
    emaskd = np.ascontiguousarray(
        np.tril(np.ones((P, P), dtype=F32)).T).astype(BF16)

    scale = 1.0 / math.sqrt(HD)
    in_maps = []
    for c in range(nc_cores):
        wq_c = np.asarray(wq, F32)[c * NQH * HD:(c + 1) * NQH * HD]  # [512, D]
        wq_c = (wq_c.reshape(NQH, HD, D)[:, perm, :] * scale).reshape(
            NQH * HD, D)
        wk_c = np.asarray(wk, F32)[c * HD:(c + 1) * HD][perm, :]  # [128, D]
        wv_c = np.asarray(wv, F32)[c * HD:(c + 1) * HD]  # [128, D]
        wqkvT = np.ascontiguousarray(
            np.concatenate([wq_c, wk_c, wv_c], axis=0).T).astype(BF16)
        woT = np.ascontiguousarray(
            np.asarray(wo, F32)[c * SLAB:(c + 1) * SLAB].T).astype(BF16)
        in_maps.append({
            "xT": xT,
            "wqkvT": wqkvT,
            "woT": woT,
            "cosq": cosq,
            "sinq": sinq,
            "emaskd": emaskd,
        })
    return in_maps


_NC_CACHE = {}


def _get_nc(nc_cores=N_CORES, s=S):
    key = (nc_cores, s)
    if key not in _NC_CACHE:
        _NC_CACHE[key] = _build(nc_cores, s)
    return _NC_CACHE[key]


def _assemble(results, nc_cores=N_CORES, s=S):
    out = np.empty((B, s, nc_cores * SLAB), dtype=F32)
    for c in range(nc_cores):
        oT = results[c]["outT"]  # [512, tok]
        out[:, :, c * SLAB:(c + 1) * SLAB] = oT.T.reshape(B, s, SLAB)
    return out


def _run(inputs, trace=False, nc_cores=N_CORES, s=S):
    from concourse.bass_utils import run_bass_kernel_spmd

    nc = _get_nc(nc_cores, s)
    in_maps = _prep_inputs(**inputs, nc_cores=nc_cores, s=s)
    res = run_bass_kernel_spmd(nc, in_maps, core_ids=list(range(nc_cores)),
                               trace=trace)
    return _assemble(res.results, nc_cores, s), res


def kernel(x, wq, wk, wv, wo, freqs_cos, freqs_sin, mask):
    out, _ = _run(dict(x=x, wq=wq, wk=wk, wv=wv, wo=wo,
                       freqs_cos=freqs_cos, freqs_sin=freqs_sin, mask=mask),
                  trace=bool(int(os.environ.get("KERNEL_TRACE", "0"))))
    return out
